# revision 13
# baseline (speedup 1.0000x reference)
"""Causal self-attention (RoPE-3D + QK-RMSNorm) on 8 TRN2 NeuronCores.

Tensor-parallel over heads: 2 heads per core. Host shards W_qkv rows /
W_out columns, replicates x (pre-transposed), precomputes fused RoPE
cos/sin tables, and sums the 8 per-core partial projection outputs.

Per-core device kernel (Bass/Tile, no collectives):
  Phase A (per 512-token block): QKV projection (fp32r matmuls),
    QK-RMSNorm via ones-matmul partition reduction + Rsqrt +
    partition_broadcast, RoPE via fused tables + stream_shuffle.
    q,k feature-major [96, tokens]; v token-major [tokens, 96(+1 ones col)].
  Phase B (per 512 q-token i-block): S^T = k_tile^T q_block (fp32r),
    exp on ACT (no max subtraction needed: |S| <= sqrt(D)), causal mask via
    affine_select, AV + softmax-denominator via [v|1] matmul (bf16),
    normalize with Reciprocal + partition_broadcast, out-projection (bf16)
    into a [C, tokens] partial that the host sums across cores.
"""

import math
from contextlib import ExitStack

import numpy as np
import ml_dtypes

import concourse.bass as bass
import concourse.mybir as mybir
import concourse.tile as tile
from concourse import bacc
from concourse.bass_utils import run_bass_kernel_spmd

B, T, C = 2, 2048, 1536
H, D = 16, 96
NT = B * T                    # 4096 tokens
NCORES = 8
HPC = H // NCORES             # heads per core
ROPE_BASE = 10000.0

F32 = mybir.dt.float32
F32R = mybir.dt.float32r
BF16 = mybir.dt.bfloat16

KT = C // 128                 # 12 contraction tiles over C
NBLK = NT // 512              # 8 token blocks
IB_PER_B = T // 512           # 4 q i-blocks per batch
VSTRIDE = 32 * 97             # v_sb per-head columns: 32 token-tiles x (96+1)

_CACHE = {}


# ----------------------------------------------------------------- host side

def _host_tables(coords, token_type, q_scale, k_scale):
    tt = (np.asarray(token_type).reshape(NT) > 0)
    half = 16
    inv_freq = ROPE_BASE ** (-np.arange(half, dtype=np.float64) / half)
    cf = np.empty((NT, D), np.float64)
    sf = np.empty((NT, D), np.float64)
    cflat = np.asarray(coords).reshape(NT, 3).astype(np.float64)
    for a in range(3):
        ang = cflat[:, a:a + 1] * inv_freq[None, :]
        c, s = np.cos(ang), np.sin(ang)
        cf[:, a * 32:a * 32 + 16] = c
        cf[:, a * 32 + 16:a * 32 + 32] = c
        sf[:, a * 32:a * 32 + 16] = -s
        sf[:, a * 32 + 16:a * 32 + 32] = s
    cf[~tt] = 1.0
    sf[~tt] = 0.0
    pi = (np.arange(D) // 32) * 32 + (np.arange(D) + 16) % 32
    c0 = 1.0 / math.sqrt(D)
    q_scale = np.asarray(q_scale, np.float64)
    k_scale = np.asarray(k_scale, np.float64)
    cosq = np.ascontiguousarray((cf * (q_scale[None, :] * c0)).T).astype(np.float32)
    sinq = np.ascontiguousarray((sf * (q_scale[pi][None, :] * c0)).T).astype(np.float32)
    cosk = np.ascontiguousarray((cf * k_scale[None, :]).T).astype(np.float32)
    sink = np.ascontiguousarray((sf * k_scale[pi][None, :]).T).astype(np.float32)
    return cosq, sinq, cosk, sink


def _make_in_maps(x, coords, token_type, W_qkv, W_out, q_scale, k_scale):
    x = np.asarray(x, np.float32)
    W_qkv = np.asarray(W_qkv, np.float32)
    W_out = np.asarray(W_out, np.float32)
    xT = np.ascontiguousarray(x.reshape(NT, C).T)
    cosq, sinq, cosk, sink = _host_tables(coords, token_type, q_scale, k_scale)
    in_maps = []
    for ci in range(NCORES):
        h0 = HPC * ci
        rows = np.concatenate([
            W_qkv[h0 * D:(h0 + HPC) * D],
            W_qkv[C + h0 * D:C + (h0 + HPC) * D],
        ], axis=0)                                        # [384, C] q,k rows
        wqkvT = np.ascontiguousarray(rows.T)              # [C, 384]
        wvT = np.ascontiguousarray(
            W_qkv[2 * C + h0 * D:2 * C + (h0 + HPC) * D].T
        ).astype(ml_dtypes.bfloat16)                      # [C, 192] bf16
        woT = np.ascontiguousarray(
            W_out[:, h0 * D:(h0 + HPC) * D].T
        ).astype(ml_dtypes.bfloat16)                      # [192, C] bf16
        in_maps.append({
            "xT": xT, "wqkvT": wqkvT, "wvT": wvT, "woT": woT,
            "onesp": np.ones((128, 1), np.float32),
            "cosq": cosq, "sinq": sinq, "cosk": cosk, "sink": sink,
        })
    return in_maps


# --------------------------------------------------------------- bass builder

SWAP16 = [(i + 16) % 32 for i in range(32)]


def _build():
    nc = bacc.Bacc("TRN2", target_bir_lowering=False, debug=False)
    AF = mybir.ActivationFunctionType

    xT = nc.declare_dram_parameter("xT", [C, NT], F32R, isOutput=False)
    wqkvT = nc.declare_dram_parameter("wqkvT", [C, 2 * HPC * D], F32R, isOutput=False)
    wvT = nc.declare_dram_parameter("wvT", [C, HPC * D], BF16, isOutput=False)
    woT = nc.declare_dram_parameter("woT", [HPC * D, C], BF16, isOutput=False)
    cosq = nc.declare_dram_parameter("cosq", [D, NT], F32, isOutput=False)
    sinq = nc.declare_dram_parameter("sinq", [D, NT], F32, isOutput=False)
    cosk = nc.declare_dram_parameter("cosk", [D, NT], F32, isOutput=False)
    sink = nc.declare_dram_parameter("sink", [D, NT], F32, isOutput=False)
    onesp = nc.declare_dram_parameter("onesp", [128, 1], F32R, isOutput=False)
    outT = nc.declare_dram_parameter("outT", [C, NT], F32, isOutput=True)

    with ExitStack() as ctx:
        tc = ctx.enter_context(tile.TileContext(nc))
        resid = ctx.enter_context(tc.tile_pool(name="resid", bufs=1))
        xp = ctx.enter_context(tc.tile_pool(name="xp", bufs=13))
        xbp = ctx.enter_context(tc.tile_pool(name="xbp", bufs=13))
        tp = ctx.enter_context(tc.tile_pool(name="tp", bufs=5))
        wk = ctx.enter_context(tc.tile_pool(name="wk", bufs=3))
        rp = ctx.enter_context(tc.tile_pool(name="rp", bufs=2))
        ep = ctx.enter_context(tc.tile_pool(name="ep", bufs=3))
        op_ = ctx.enter_context(tc.tile_pool(name="op", bufs=2))
        obp = ctx.enter_context(tc.tile_pool(name="obp", bufs=2))
        ps = ctx.enter_context(tc.tile_pool(name="ps", bufs=2, space="PSUM"))

        # ---- residents
        wq_sb = resid.tile([128, KT * 384], F32R, tag="wq")
        nc.sync.dma_start(
            out=wq_sb[:].rearrange("p (a f) -> p a f", a=KT),
            in_=wqkvT[:, :].rearrange("(a p) f -> p a f", p=128),
        )
        wv_sb = resid.tile([128, KT * 192], BF16, tag="wv")
        nc.sync.dma_start(
            out=wv_sb[:].rearrange("p (a f) -> p a f", a=KT),
            in_=wvT[:, :].rearrange("(a p) f -> p a f", p=128),
        )
        wo_sb = resid.tile([96, HPC * C], BF16, tag="wo")
        nc.sync.dma_start(
            out=wo_sb[:].rearrange("p (h f) -> p h f", h=HPC),
            in_=woT[:, :].rearrange("(h p) f -> p h f", p=96),
        )
        qT_sb = resid.tile([96, HPC * NT], F32R, tag="qT")
        kT_sb = resid.tile([96, HPC * NT], F32R, tag="kT")
        v_sb = resid.tile([128, HPC * VSTRIDE], BF16, tag="v")
        ones_sb = resid.tile([128, 1], F32R, tag="ones")
        nc.sync.dma_start(out=ones_sb[:], in_=onesp[:, :])
        eps_sb = resid.tile([1, 1], F32, tag="eps")
        nc.gpsimd.memset(eps_sb[:], 1e-6)
        # ones column of each v token-tile
        nc.gpsimd.memset(
            v_sb[:].rearrange("p (h t f) -> p h t f", h=HPC, t=32)[:, :, :, 96:97],
            1.0,
        )

        tabs = {"q": (cosq, sinq), "k": (cosk, sink)}

        def phase_a(n):
            """QKV + norm + rope for token block n (512 tokens)."""
            xt = []
            xb = []
            for kt in range(KT):
                t = xp.tile([128, 512], F32R, tag="xt")
                nc.sync.dma_start(
                    out=t[:], in_=xT[kt * 128:(kt + 1) * 128, n * 512:(n + 1) * 512])
                xt.append(t)
                tb = xbp.tile([128, 512], BF16, tag="xb")
                nc.vector.tensor_copy(tb[:], t[:].bitcast(F32))
                xb.append(tb)
            # q0,q1,k0,k1 feature-major
            for g in range(2 * HPC):
                qk = "q" if g < HPC else "k"
                hh = g % HPC
                wcol = g * 96
                pa = ps.tile([96, 512], F32, tag="pA")
                for kt in range(KT):
                    nc.tensor.matmul(
                        pa[:],
                        lhsT=wq_sb[:, kt * 384 + wcol:kt * 384 + wcol + 96],
                        rhs=xt[kt][:],
                        start=(kt == 0), stop=(kt == KT - 1),
                    )
                sq = wk.tile([96, 512], F32R, tag="qn")
                nc.scalar.activation(sq[:], pa[:], AF.Square)
                ssq = ps.tile([1, 512], F32, tag="pS")
                nc.tensor.matmul(
                    ssq[:], lhsT=ones_sb[0:96, 0:1],
                    rhs=sq[:], start=True, stop=True,
                )
                rnorm = rp.tile([1, 512], F32, tag="row")
                nc.scalar.activation(rnorm[:], ssq[:], AF.Sqrt,
                                     scale=1.0 / D, bias=eps_sb[:])
                rinv = rp.tile([1, 512], F32, tag="row")
                nc.vector.reciprocal(rinv[:], rnorm[:])
                binv = wk.tile([96, 512], F32, tag="qs")
                nc.gpsimd.partition_broadcast(binv[:], rinv[:])
                qn = wk.tile([96, 512], F32, tag="qn")
                nc.vector.tensor_mul(qn[:], pa[:], binv[:])
                qs = wk.tile([96, 512], F32, tag="qs")
                nc.vector.stream_shuffle(qs[:], qn[:], mask=SWAP16)
                ctab, stab = tabs[qk]
                tco = tp.tile([96, 512], F32, tag="tbl")
                nc.sync.dma_start(out=tco[:], in_=ctab[0:96, n * 512:(n + 1) * 512])
                tsi = tp.tile([96, 512], F32, tag="tbl")
                nc.sync.dma_start(out=tsi[:], in_=stab[0:96, n * 512:(n + 1) * 512])
                nc.vector.tensor_mul(qn[:], qn[:], tco[:])
                nc.vector.tensor_mul(qs[:], qs[:], tsi[:])
                dest = (qT_sb if qk == "q" else kT_sb)
                nc.vector.tensor_add(
                    dest[:, hh * NT + n * 512:hh * NT + (n + 1) * 512], qn[:], qs[:])
            # v token-major (bf16)
            for m in range(4):
                pv = ps.tile([128, HPC * 96], F32, tag="pV")
                for kt in range(KT):
                    nc.tensor.matmul(
                        pv[:],
                        lhsT=xb[kt][:, m * 128:(m + 1) * 128],
                        rhs=wv_sb[:, kt * 192:(kt + 1) * 192],
                        start=(kt == 0), stop=(kt == KT - 1),
                    )
                tt = n * 4 + m
                for hh in range(HPC):
                    nc.scalar.activation(
                        v_sb[:, hh * VSTRIDE + tt * 97:hh * VSTRIDE + tt * 97 + 96],
                        pv[:, hh * 96:(hh + 1) * 96], AF.Copy)

        def phase_b(b, ib):
            """Attention + out-projection for q i-block ib of batch b."""
            tok0 = b * T + ib * 512
            ous = []
            for hh in range(HPC):
                ups = ps.tile([97, 512], F32, tag="pS")
                njt = 4 * ib + 4
                for jt in range(njt):
                    sps = ps.tile([128, 512], F32, tag="pA")
                    jtok = b * T + jt * 128
                    nc.tensor.matmul(
                        sps[:],
                        lhsT=kT_sb[:, hh * NT + jtok:hh * NT + jtok + 128],
                        rhs=qT_sb[:, hh * NT + tok0:hh * NT + tok0 + 512],
                        start=True, stop=True,
                    )
                    es = ep.tile([128, 512], BF16, tag="es")
                    nc.scalar.activation(es[:], sps[:], AF.Exp)
                    if jt >= 4 * ib:
                        s = jt - 4 * ib
                        # keep where (q idx) i - 128*s - j >= 0
                        nc.gpsimd.affine_select(
                            out=es[:], in_=es[:],
                            compare_op=mybir.AluOpType.is_ge,
                            fill=0.0, base=-128 * s, channel_multiplier=-1,
                            pattern=[[1, 512]],
                        )
                    gt = b * 16 + jt
                    nc.tensor.matmul(
                        ups[:],
                        lhsT=v_sb[:, hh * VSTRIDE + gt * 97:hh * VSTRIDE + gt * 97 + 97],
                        rhs=es[:],
                        start=(jt == 0), stop=(jt == njt - 1),
                    )
                zinv = rp.tile([1, 512], F32, tag="row")
                nc.vector.reciprocal(zinv[:], ups[96:97, :])
                zb = wk.tile([96, 512], F32, tag="qs")
                nc.gpsimd.partition_broadcast(zb[:], zinv[:])
                ou = op_.tile([96, 512], BF16, tag=f"ou{hh}")
                nc.vector.tensor_mul(ou[:], ups[0:96, :], zb[:])
                ous.append(ou)
            for ct in range(KT):
                ops = ps.tile([128, 512], F32, tag="pV")
                for hh in range(HPC):
                    nc.tensor.matmul(
                        ops[:],
                        lhsT=wo_sb[:, hh * C + ct * 128:hh * C + ct * 128 + 128],
                        rhs=ous[hh][:],
                        start=(hh == 0), stop=(hh == HPC - 1),
                    )
                ob = obp.tile([128, 512], F32, tag="ob")
                nc.any.tensor_copy(ob[:], ops[:])
                nc.sync.dma_start(
                    out=outT[ct * 128:(ct + 1) * 128, tok0:tok0 + 512], in_=ob[:])

        # interleave: block n's phase A, then the i-block whose deps it closes
        for n in range(NBLK):
            phase_a(n)
            b, ib = divmod(n, IB_PER_B)
            phase_b(b, ib)

    nc.compile()
    return nc


def _get_nc():
    if "nc" not in _CACHE:
        _CACHE["nc"] = _build()
    return _CACHE["nc"]


# ------------------------------------------------------------------ entrypoint

def _run(inputs, trace=False, **kw):
    nc = _get_nc()
    in_maps = _make_in_maps(**inputs)
    res = run_bass_kernel_spmd(nc, in_maps, core_ids=list(range(NCORES)),
                               trace=trace, **kw)
    acc = np.zeros((C, NT), np.float64)
    for r in res.results:
        acc += r["outT"].astype(np.float64)
    out = np.ascontiguousarray(acc.T.astype(np.float32)).reshape(B, T, C)
    return out, res


def kernel(**inputs) -> np.ndarray:
    out, _ = _run(inputs, trace=False)
    return out


# revision 15
# speedup vs baseline: 1.0737x; 1.0737x over previous
"""Causal self-attention (RoPE-3D + QK-RMSNorm) on 8 TRN2 NeuronCores.

Tensor-parallel over heads: 2 heads per core. Host shards W_qkv rows /
W_out columns, replicates x (pre-transposed), precomputes fused RoPE
cos/sin tables, and sums the 8 per-core partial projection outputs.

Per-core device kernel (Bass/Tile, no collectives):
  Phase A (per 512-token block): QKV projection (fp32r matmuls),
    QK-RMSNorm via ones-matmul partition reduction + Rsqrt +
    partition_broadcast, RoPE via fused tables + stream_shuffle.
    q,k feature-major [96, tokens]; v token-major [tokens, 96(+1 ones col)].
  Phase B (per 512 q-token i-block): S^T = k_tile^T q_block (fp32r),
    exp on ACT (no max subtraction needed: |S| <= sqrt(D)), causal mask via
    affine_select, AV + softmax-denominator via [v|1] matmul (bf16),
    normalize with Reciprocal + partition_broadcast, out-projection (bf16)
    into a [C, tokens] partial that the host sums across cores.
"""

import math
from contextlib import ExitStack

import numpy as np
import ml_dtypes

import concourse.bass as bass
import concourse.mybir as mybir
import concourse.tile as tile
from concourse import bacc
from concourse.bass_utils import run_bass_kernel_spmd

B, T, C = 2, 2048, 1536
H, D = 16, 96
NT = B * T                    # 4096 tokens
NCORES = 8
HPC = H // NCORES             # heads per core
ROPE_BASE = 10000.0

F32 = mybir.dt.float32
F32R = mybir.dt.float32r
BF16 = mybir.dt.bfloat16

KT = C // 128                 # 12 contraction tiles over C
NBLK = NT // 512              # 8 token blocks
IB_PER_B = T // 512           # 4 q i-blocks per batch
VSTRIDE = 32 * 97             # v_sb per-head columns: 32 token-tiles x (96+1)

_CACHE = {}


# ----------------------------------------------------------------- host side

def _host_tables(coords, token_type, q_scale, k_scale):
    tt = (np.asarray(token_type).reshape(NT) > 0)
    half = 16
    inv_freq = ROPE_BASE ** (-np.arange(half, dtype=np.float64) / half)
    cf = np.empty((NT, D), np.float64)
    sf = np.empty((NT, D), np.float64)
    cflat = np.asarray(coords).reshape(NT, 3).astype(np.float64)
    for a in range(3):
        ang = cflat[:, a:a + 1] * inv_freq[None, :]
        c, s = np.cos(ang), np.sin(ang)
        cf[:, a * 32:a * 32 + 16] = c
        cf[:, a * 32 + 16:a * 32 + 32] = c
        sf[:, a * 32:a * 32 + 16] = -s
        sf[:, a * 32 + 16:a * 32 + 32] = s
    cf[~tt] = 1.0
    sf[~tt] = 0.0
    pi = (np.arange(D) // 32) * 32 + (np.arange(D) + 16) % 32
    c0 = 1.0 / math.sqrt(D)
    q_scale = np.asarray(q_scale, np.float64)
    k_scale = np.asarray(k_scale, np.float64)
    cosq = np.ascontiguousarray((cf * (q_scale[None, :] * c0)).T).astype(np.float32)
    sinq = np.ascontiguousarray((sf * (q_scale[pi][None, :] * c0)).T).astype(np.float32)
    cosk = np.ascontiguousarray((cf * k_scale[None, :]).T).astype(np.float32)
    sink = np.ascontiguousarray((sf * k_scale[pi][None, :]).T).astype(np.float32)
    return cosq, sinq, cosk, sink


def _make_in_maps(x, coords, token_type, W_qkv, W_out, q_scale, k_scale):
    x = np.asarray(x, np.float32)
    W_qkv = np.asarray(W_qkv, np.float32)
    W_out = np.asarray(W_out, np.float32)
    xT = np.ascontiguousarray(x.reshape(NT, C).T)
    xbT = xT.astype(ml_dtypes.bfloat16)
    cosq, sinq, cosk, sink = _host_tables(coords, token_type, q_scale, k_scale)
    in_maps = []
    for ci in range(NCORES):
        h0 = HPC * ci
        rows = np.concatenate([
            W_qkv[h0 * D:(h0 + HPC) * D],
            W_qkv[C + h0 * D:C + (h0 + HPC) * D],
        ], axis=0)                                        # [384, C] q,k rows
        wqkvT = np.ascontiguousarray(rows.T)              # [C, 384]
        wvT = np.ascontiguousarray(
            W_qkv[2 * C + h0 * D:2 * C + (h0 + HPC) * D].T
        ).astype(ml_dtypes.bfloat16)                      # [C, 192] bf16
        woT = np.ascontiguousarray(
            W_out[:, h0 * D:(h0 + HPC) * D].T
        ).astype(ml_dtypes.bfloat16)                      # [192, C] bf16
        in_maps.append({
            "xT": xT, "wqkvT": wqkvT, "wvT": wvT, "woT": woT,
            "onesp": np.ones((128, 1), np.float32), "xbT": xbT,
            "cosq": cosq, "sinq": sinq, "cosk": cosk, "sink": sink,
        })
    return in_maps


# --------------------------------------------------------------- bass builder

SWAP16 = [(i + 16) % 32 for i in range(32)]


def _build():
    nc = bacc.Bacc("TRN2", target_bir_lowering=False, debug=False)
    AF = mybir.ActivationFunctionType

    xT = nc.declare_dram_parameter("xT", [C, NT], F32R, isOutput=False)
    xbT = nc.declare_dram_parameter("xbT", [C, NT], BF16, isOutput=False)
    wqkvT = nc.declare_dram_parameter("wqkvT", [C, 2 * HPC * D], F32R, isOutput=False)
    wvT = nc.declare_dram_parameter("wvT", [C, HPC * D], BF16, isOutput=False)
    woT = nc.declare_dram_parameter("woT", [HPC * D, C], BF16, isOutput=False)
    cosq = nc.declare_dram_parameter("cosq", [D, NT], F32, isOutput=False)
    sinq = nc.declare_dram_parameter("sinq", [D, NT], F32, isOutput=False)
    cosk = nc.declare_dram_parameter("cosk", [D, NT], F32, isOutput=False)
    sink = nc.declare_dram_parameter("sink", [D, NT], F32, isOutput=False)
    onesp = nc.declare_dram_parameter("onesp", [128, 1], F32R, isOutput=False)
    outT = nc.declare_dram_parameter("outT", [C, NT], BF16, isOutput=True)

    with ExitStack() as ctx:
        tc = ctx.enter_context(tile.TileContext(nc))
        resid = ctx.enter_context(tc.tile_pool(name="resid", bufs=1))
        xp = ctx.enter_context(tc.tile_pool(name="xp", bufs=13))
        xbp = ctx.enter_context(tc.tile_pool(name="xbp", bufs=13))
        tp = ctx.enter_context(tc.tile_pool(name="tp", bufs=5))
        wk = ctx.enter_context(tc.tile_pool(name="wk", bufs=3))
        rp = ctx.enter_context(tc.tile_pool(name="rp", bufs=3))
        ep = ctx.enter_context(tc.tile_pool(name="ep", bufs=4))
        op_ = ctx.enter_context(tc.tile_pool(name="op", bufs=2))
        obp = ctx.enter_context(tc.tile_pool(name="obp", bufs=2))
        ps = ctx.enter_context(tc.tile_pool(name="ps", bufs=2, space="PSUM"))
        ps3 = ctx.enter_context(tc.tile_pool(name="ps3", bufs=3, space="PSUM"))

        # ---- residents
        wq_sb = resid.tile([128, KT * 384], F32R, tag="wq")
        nc.sync.dma_start(
            out=wq_sb[:].rearrange("p (a f) -> p a f", a=KT),
            in_=wqkvT[:, :].rearrange("(a p) f -> p a f", p=128),
        )
        wv_sb = resid.tile([128, KT * 192], BF16, tag="wv")
        nc.sync.dma_start(
            out=wv_sb[:].rearrange("p (a f) -> p a f", a=KT),
            in_=wvT[:, :].rearrange("(a p) f -> p a f", p=128),
        )
        wo_sb = resid.tile([96, HPC * C], BF16, tag="wo")
        nc.sync.dma_start(
            out=wo_sb[:].rearrange("p (h f) -> p h f", h=HPC),
            in_=woT[:, :].rearrange("(h p) f -> p h f", p=96),
        )
        qT_sb = resid.tile([96, HPC * NT], F32R, tag="qT")
        kT_sb = resid.tile([96, HPC * NT], F32R, tag="kT")
        v_sb = resid.tile([128, HPC * VSTRIDE], BF16, tag="v")
        ones_sb = resid.tile([128, 1], F32R, tag="ones")
        nc.sync.dma_start(out=ones_sb[:], in_=onesp[:, :])
        eps_sb = resid.tile([1, 1], F32, tag="eps")
        nc.gpsimd.memset(eps_sb[:], 1e-6)
        # ones column of each v token-tile
        nc.gpsimd.memset(
            v_sb[:].rearrange("p (h t f) -> p h t f", h=HPC, t=32)[:, :, :, 96:97],
            1.0,
        )

        tabs = {"q": (cosq, sinq), "k": (cosk, sink)}

        def phase_a(n):
            """QKV + norm + rope for token block n (512 tokens)."""
            xt = []
            xb = []
            for kt in range(KT):
                t = xp.tile([128, 512], F32R, tag="xt")
                nc.sync.dma_start(
                    out=t[:], in_=xT[kt * 128:(kt + 1) * 128, n * 512:(n + 1) * 512])
                xt.append(t)
                tb = xbp.tile([128, 512], BF16, tag="xb")
                nc.sync.dma_start(
                    out=tb[:], in_=xbT[kt * 128:(kt + 1) * 128, n * 512:(n + 1) * 512])
                xb.append(tb)
            # q0,q1,k0,k1 feature-major
            for g in range(2 * HPC):
                qk = "q" if g < HPC else "k"
                hh = g % HPC
                wcol = g * 96
                pa = ps3.tile([96, 512], F32, tag="pA")
                for kt in range(KT):
                    nc.tensor.matmul(
                        pa[:],
                        lhsT=wq_sb[:, kt * 384 + wcol:kt * 384 + wcol + 96],
                        rhs=xt[kt][:],
                        start=(kt == 0), stop=(kt == KT - 1),
                    )
                sq = wk.tile([96, 512], F32R, tag="qn")
                nc.scalar.activation(sq[:], pa[:], AF.Square)
                ssq = ps.tile([1, 512], F32, tag="pS")
                nc.tensor.matmul(
                    ssq[:], lhsT=ones_sb[0:96, 0:1],
                    rhs=sq[:], start=True, stop=True,
                )
                rinv = rp.tile([1, 512], F32, tag="row")
                nc.scalar.activation(rinv[:], ssq[:], AF.Abs_reciprocal_sqrt,
                                     scale=1.0 / D, bias=eps_sb[:])
                binv = wk.tile([96, 512], F32, tag="qs")
                nc.gpsimd.partition_broadcast(binv[:], rinv[:])
                qn = wk.tile([96, 512], F32, tag="qn")
                nc.vector.tensor_mul(qn[:], pa[:], binv[:])
                qs = wk.tile([96, 512], F32, tag="qs")
                nc.vector.stream_shuffle(qs[:], qn[:], mask=SWAP16)
                ctab, stab = tabs[qk]
                tco = tp.tile([96, 512], F32, tag="tbl")
                nc.sync.dma_start(out=tco[:], in_=ctab[0:96, n * 512:(n + 1) * 512])
                tsi = tp.tile([96, 512], F32, tag="tbl")
                nc.sync.dma_start(out=tsi[:], in_=stab[0:96, n * 512:(n + 1) * 512])
                nc.vector.tensor_mul(qn[:], qn[:], tco[:])
                nc.vector.tensor_mul(qs[:], qs[:], tsi[:])
                dest = (qT_sb if qk == "q" else kT_sb)
                nc.vector.tensor_add(
                    dest[:, hh * NT + n * 512:hh * NT + (n + 1) * 512], qn[:], qs[:])
            # v token-major (bf16)
            for m in range(4):
                pv = ps.tile([128, HPC * 96], F32, tag="pV")
                for kt in range(KT):
                    nc.tensor.matmul(
                        pv[:],
                        lhsT=xb[kt][:, m * 128:(m + 1) * 128],
                        rhs=wv_sb[:, kt * 192:(kt + 1) * 192],
                        start=(kt == 0), stop=(kt == KT - 1),
                    )
                tt = n * 4 + m
                for hh in range(HPC):
                    nc.vector.tensor_copy(
                        v_sb[:, hh * VSTRIDE + tt * 97:hh * VSTRIDE + tt * 97 + 96],
                        pv[:, hh * 96:(hh + 1) * 96])

        def phase_b(b, ib):
            """Attention + out-projection for q i-block ib of batch b."""
            tok0 = b * T + ib * 512
            ous = []
            for hh in range(HPC):
                ups = ps.tile([97, 512], F32, tag="pS")
                njt = 4 * ib + 4
                for jt in range(njt):
                    sps = ps3.tile([128, 512], F32, tag="pA")
                    jtok = b * T + jt * 128
                    nc.tensor.matmul(
                        sps[:],
                        lhsT=kT_sb[:, hh * NT + jtok:hh * NT + jtok + 128],
                        rhs=qT_sb[:, hh * NT + tok0:hh * NT + tok0 + 512],
                        start=True, stop=True,
                    )
                    es = ep.tile([128, 512], BF16, tag="es")
                    nc.scalar.activation(es[:], sps[:], AF.Exp)
                    if jt >= 4 * ib:
                        s = jt - 4 * ib
                        # keep where (q idx) i - 128*s - j >= 0
                        nc.gpsimd.affine_select(
                            out=es[:], in_=es[:],
                            compare_op=mybir.AluOpType.is_ge,
                            fill=0.0, base=-128 * s, channel_multiplier=-1,
                            pattern=[[1, 512]],
                        )
                    gt = b * 16 + jt
                    nc.tensor.matmul(
                        ups[:],
                        lhsT=v_sb[:, hh * VSTRIDE + gt * 97:hh * VSTRIDE + gt * 97 + 97],
                        rhs=es[:],
                        start=(jt == 0), stop=(jt == njt - 1),
                    )
                zrs = rp.tile([1, 512], F32, tag="row")
                nc.scalar.activation(zrs[:], ups[96:97, :], AF.Abs_reciprocal_sqrt)
                zinv = rp.tile([1, 512], F32, tag="row")
                nc.scalar.activation(zinv[:], zrs[:], AF.Square)
                zb = wk.tile([96, 512], F32, tag="qs")
                nc.gpsimd.partition_broadcast(zb[:], zinv[:])
                ou = op_.tile([96, 512], BF16, tag=f"ou{hh}")
                nc.vector.tensor_mul(ou[:], ups[0:96, :], zb[:])
                ous.append(ou)
            for ct in range(KT):
                ops = ps.tile([128, 512], F32, tag="pV")
                for hh in range(HPC):
                    nc.tensor.matmul(
                        ops[:],
                        lhsT=wo_sb[:, hh * C + ct * 128:hh * C + ct * 128 + 128],
                        rhs=ous[hh][:],
                        start=(hh == 0), stop=(hh == HPC - 1),
                    )
                ob = obp.tile([128, 512], BF16, tag="ob")
                nc.vector.tensor_copy(ob[:], ops[:])
                nc.sync.dma_start(
                    out=outT[ct * 128:(ct + 1) * 128, tok0:tok0 + 512], in_=ob[:])

        # interleave: block n's phase A, then the i-block whose deps it closes
        for n in range(NBLK):
            phase_a(n)
            b, ib = divmod(n, IB_PER_B)
            phase_b(b, ib)

    nc.compile()
    return nc


def _get_nc():
    if "nc" not in _CACHE:
        _CACHE["nc"] = _build()
    return _CACHE["nc"]


# ------------------------------------------------------------------ entrypoint

def _run(inputs, trace=False, **kw):
    nc = _get_nc()
    in_maps = _make_in_maps(**inputs)
    res = run_bass_kernel_spmd(nc, in_maps, core_ids=list(range(NCORES)),
                               trace=trace, **kw)
    acc = np.zeros((C, NT), np.float64)
    for r in res.results:
        acc += r["outT"].astype(np.float64)
    out = np.ascontiguousarray(acc.T.astype(np.float32)).reshape(B, T, C)
    return out, res


def kernel(**inputs) -> np.ndarray:
    out, _ = _run(inputs, trace=False)
    return out


# revision 16
# speedup vs baseline: 1.0931x; 1.0181x over previous
"""Causal self-attention (RoPE-3D + QK-RMSNorm) on 8 TRN2 NeuronCores.

Tensor-parallel over heads: 2 heads per core. Host shards W_qkv rows /
W_out columns, replicates x (pre-transposed), precomputes fused RoPE
cos/sin tables, and sums the 8 per-core partial projection outputs.

Per-core device kernel (Bass/Tile, no collectives):
  Phase A (per 512-token block): QKV projection (fp32r matmuls),
    QK-RMSNorm via ones-matmul partition reduction + Rsqrt +
    partition_broadcast, RoPE via fused tables + stream_shuffle.
    q,k feature-major [96, tokens]; v token-major [tokens, 96(+1 ones col)].
  Phase B (per 512 q-token i-block): S^T = k_tile^T q_block (fp32r),
    exp on ACT (no max subtraction needed: |S| <= sqrt(D)), causal mask via
    affine_select, AV + softmax-denominator via [v|1] matmul (bf16),
    normalize with Reciprocal + partition_broadcast, out-projection (bf16)
    into a [C, tokens] partial that the host sums across cores.
"""

import math
from contextlib import ExitStack

import numpy as np
import ml_dtypes

import concourse.bass as bass
import concourse.mybir as mybir
import concourse.tile as tile
from concourse import bacc
from concourse.bass_utils import run_bass_kernel_spmd

B, T, C = 2, 2048, 1536
H, D = 16, 96
NT = B * T                    # 4096 tokens
NCORES = 8
HPC = H // NCORES             # heads per core
ROPE_BASE = 10000.0

F32 = mybir.dt.float32
F32R = mybir.dt.float32r
BF16 = mybir.dt.bfloat16

KT = C // 128                 # 12 contraction tiles over C
NBLK = NT // 512              # 8 token blocks
IB_PER_B = T // 512           # 4 q i-blocks per batch
VSTRIDE = 32 * 97             # v_sb per-head columns: 32 token-tiles x (96+1)

_CACHE = {}


# ----------------------------------------------------------------- host side

def _host_tables(coords, token_type, q_scale, k_scale):
    tt = (np.asarray(token_type).reshape(NT) > 0)
    half = 16
    inv_freq = ROPE_BASE ** (-np.arange(half, dtype=np.float64) / half)
    cf = np.empty((NT, D), np.float64)
    sf = np.empty((NT, D), np.float64)
    cflat = np.asarray(coords).reshape(NT, 3).astype(np.float64)
    for a in range(3):
        ang = cflat[:, a:a + 1] * inv_freq[None, :]
        c, s = np.cos(ang), np.sin(ang)
        cf[:, a * 32:a * 32 + 16] = c
        cf[:, a * 32 + 16:a * 32 + 32] = c
        sf[:, a * 32:a * 32 + 16] = -s
        sf[:, a * 32 + 16:a * 32 + 32] = s
    cf[~tt] = 1.0
    sf[~tt] = 0.0
    pi = (np.arange(D) // 32) * 32 + (np.arange(D) + 16) % 32
    c0 = 1.0 / math.sqrt(D)
    q_scale = np.asarray(q_scale, np.float64)
    k_scale = np.asarray(k_scale, np.float64)
    cosq = np.ascontiguousarray((cf * (q_scale[None, :] * c0)).T).astype(np.float32)
    sinq = np.ascontiguousarray((sf * (q_scale[pi][None, :] * c0)).T).astype(np.float32)
    cosk = np.ascontiguousarray((cf * k_scale[None, :]).T).astype(np.float32)
    sink = np.ascontiguousarray((sf * k_scale[pi][None, :]).T).astype(np.float32)
    return cosq, sinq, cosk, sink


def _make_in_maps(x, coords, token_type, W_qkv, W_out, q_scale, k_scale):
    x = np.asarray(x, np.float32)
    W_qkv = np.asarray(W_qkv, np.float32)
    W_out = np.asarray(W_out, np.float32)
    xT = np.ascontiguousarray(x.reshape(NT, C).T)
    xbT = xT.astype(ml_dtypes.bfloat16)
    cosq, sinq, cosk, sink = _host_tables(coords, token_type, q_scale, k_scale)
    in_maps = []
    for ci in range(NCORES):
        h0 = HPC * ci
        rows = np.concatenate([
            W_qkv[h0 * D:(h0 + HPC) * D],
            W_qkv[C + h0 * D:C + (h0 + HPC) * D],
        ], axis=0)                                        # [384, C] q,k rows
        wqkvT = np.ascontiguousarray(rows.T)              # [C, 384]
        wvT = np.ascontiguousarray(
            W_qkv[2 * C + h0 * D:2 * C + (h0 + HPC) * D].T
        ).astype(ml_dtypes.bfloat16)                      # [C, 192] bf16
        woT = np.ascontiguousarray(
            W_out[:, h0 * D:(h0 + HPC) * D].T
        ).astype(ml_dtypes.bfloat16)                      # [192, C] bf16
        in_maps.append({
            "xT": xT, "wqkvT": wqkvT, "wvT": wvT, "woT": woT,
            "onesp": np.ones((128, 1), np.float32), "xbT": xbT,
            "ones96p": np.ones((1, 96), np.float32),
            "cosq": cosq, "sinq": sinq, "cosk": cosk, "sink": sink,
        })
    return in_maps


# --------------------------------------------------------------- bass builder

SWAP16 = [(i + 16) % 32 for i in range(32)]


def _build():
    nc = bacc.Bacc("TRN2", target_bir_lowering=False, debug=False)
    AF = mybir.ActivationFunctionType

    xT = nc.declare_dram_parameter("xT", [C, NT], F32R, isOutput=False)
    xbT = nc.declare_dram_parameter("xbT", [C, NT], BF16, isOutput=False)
    wqkvT = nc.declare_dram_parameter("wqkvT", [C, 2 * HPC * D], F32R, isOutput=False)
    wvT = nc.declare_dram_parameter("wvT", [C, HPC * D], BF16, isOutput=False)
    woT = nc.declare_dram_parameter("woT", [HPC * D, C], BF16, isOutput=False)
    cosq = nc.declare_dram_parameter("cosq", [D, NT], F32, isOutput=False)
    sinq = nc.declare_dram_parameter("sinq", [D, NT], F32, isOutput=False)
    cosk = nc.declare_dram_parameter("cosk", [D, NT], F32, isOutput=False)
    sink = nc.declare_dram_parameter("sink", [D, NT], F32, isOutput=False)
    onesp = nc.declare_dram_parameter("onesp", [128, 1], F32R, isOutput=False)
    ones96p = nc.declare_dram_parameter("ones96p", [1, 96], F32R, isOutput=False)
    outT = nc.declare_dram_parameter("outT", [C, NT], BF16, isOutput=True)

    with ExitStack() as ctx:
        tc = ctx.enter_context(tile.TileContext(nc))
        resid = ctx.enter_context(tc.tile_pool(name="resid", bufs=1))
        xp = ctx.enter_context(tc.tile_pool(name="xp", bufs=13))
        xbp = ctx.enter_context(tc.tile_pool(name="xbp", bufs=13))
        tp = ctx.enter_context(tc.tile_pool(name="tp", bufs=5))
        wk = ctx.enter_context(tc.tile_pool(name="wk", bufs=4))
        rp = ctx.enter_context(tc.tile_pool(name="rp", bufs=3))
        ep = ctx.enter_context(tc.tile_pool(name="ep", bufs=4))
        op_ = ctx.enter_context(tc.tile_pool(name="op", bufs=2))
        obp = ctx.enter_context(tc.tile_pool(name="obp", bufs=2))
        ps = ctx.enter_context(tc.tile_pool(name="ps", bufs=2, space="PSUM"))
        ps3 = ctx.enter_context(tc.tile_pool(name="ps3", bufs=3, space="PSUM"))

        # ---- residents
        wq_sb = resid.tile([128, KT * 384], F32R, tag="wq")
        nc.sync.dma_start(
            out=wq_sb[:].rearrange("p (a f) -> p a f", a=KT),
            in_=wqkvT[:, :].rearrange("(a p) f -> p a f", p=128),
        )
        wv_sb = resid.tile([128, KT * 192], BF16, tag="wv")
        nc.sync.dma_start(
            out=wv_sb[:].rearrange("p (a f) -> p a f", a=KT),
            in_=wvT[:, :].rearrange("(a p) f -> p a f", p=128),
        )
        wo_sb = resid.tile([96, HPC * C], BF16, tag="wo")
        nc.sync.dma_start(
            out=wo_sb[:].rearrange("p (h f) -> p h f", h=HPC),
            in_=woT[:, :].rearrange("(h p) f -> p h f", p=96),
        )
        qT_sb = resid.tile([96, HPC * NT], F32R, tag="qT")
        kT_sb = resid.tile([96, HPC * NT], F32R, tag="kT")
        v_sb = resid.tile([128, HPC * VSTRIDE], BF16, tag="v")
        ones_sb = resid.tile([128, 1], F32R, tag="ones")
        nc.sync.dma_start(out=ones_sb[:], in_=onesp[:, :])
        ones96_sb = resid.tile([1, 96], F32R, tag="ones96")
        nc.sync.dma_start(out=ones96_sb[:], in_=ones96p[:, :])
        eps_sb = resid.tile([1, 1], F32, tag="eps")
        nc.gpsimd.memset(eps_sb[:], 1e-6)
        # ones column of each v token-tile
        nc.gpsimd.memset(
            v_sb[:].rearrange("p (h t f) -> p h t f", h=HPC, t=32)[:, :, :, 96:97],
            1.0,
        )

        tabs = {"q": (cosq, sinq), "k": (cosk, sink)}

        def phase_a(n):
            """QKV + norm + rope for token block n (512 tokens)."""
            xt = []
            xb = []
            for kt in range(KT):
                t = xp.tile([128, 512], F32R, tag="xt")
                nc.sync.dma_start(
                    out=t[:], in_=xT[kt * 128:(kt + 1) * 128, n * 512:(n + 1) * 512])
                xt.append(t)
                tb = xbp.tile([128, 512], BF16, tag="xb")
                nc.sync.dma_start(
                    out=tb[:], in_=xbT[kt * 128:(kt + 1) * 128, n * 512:(n + 1) * 512])
                xb.append(tb)
            # q0,q1,k0,k1 feature-major: two passes so ACT batches its
            # Square and Abs_reciprocal_sqrt table loads.
            pas = []
            for g in range(2 * HPC):
                wcol = g * 96
                pa = ps3.tile([96, 512], F32, tag="pA")
                for kt in range(KT):
                    nc.tensor.matmul(
                        pa[:],
                        lhsT=wq_sb[:, kt * 384 + wcol:kt * 384 + wcol + 96],
                        rhs=xt[kt][:],
                        start=(kt == 0), stop=(kt == KT - 1),
                    )
                sq = wk.tile([96, 512], F32R, tag="sq")
                nc.scalar.activation(sq[:], pa[:], AF.Square)
                pas.append((pa, sq))
            for g in range(2 * HPC):
                qk = "q" if g < HPC else "k"
                hh = g % HPC
                pa, sq = pas[g]
                ssq = ps.tile([1, 512], F32, tag="pS")
                nc.tensor.matmul(
                    ssq[:], lhsT=ones_sb[0:96, 0:1],
                    rhs=sq[:], start=True, stop=True,
                )
                rinv = rp.tile([1, 512], F32R, tag="row")
                nc.scalar.activation(rinv[:], ssq[:], AF.Abs_reciprocal_sqrt,
                                     scale=1.0 / D, bias=eps_sb[:])
                bq = ps.tile([96, 512], F32, tag="pV")
                nc.tensor.matmul(bq[:], lhsT=ones96_sb[:, :], rhs=rinv[:],
                                 start=True, stop=True)
                ctab, stab = tabs[qk]
                tco = tp.tile([96, 512], F32, tag="tbl")
                nc.sync.dma_start(out=tco[:], in_=ctab[0:96, n * 512:(n + 1) * 512])
                tsi = tp.tile([96, 512], F32, tag="tbl")
                nc.sync.dma_start(out=tsi[:], in_=stab[0:96, n * 512:(n + 1) * 512])
                m1 = wk.tile([96, 512], F32, tag="m1")
                nc.vector.tensor_mul(m1[:], pa[:], tco[:])
                qsh = wk.tile([96, 512], F32, tag="qsh")
                nc.vector.stream_shuffle(qsh[:], pa[:], mask=SWAP16)
                nc.vector.tensor_mul(qsh[:], qsh[:], tsi[:])
                nc.vector.tensor_add(m1[:], m1[:], qsh[:])
                dest = (qT_sb if qk == "q" else kT_sb)
                nc.vector.tensor_mul(
                    dest[:, hh * NT + n * 512:hh * NT + (n + 1) * 512], m1[:], bq[:])
            # v token-major (bf16)
            for m in range(4):
                pv = ps.tile([128, HPC * 96], F32, tag="pV")
                for kt in range(KT):
                    nc.tensor.matmul(
                        pv[:],
                        lhsT=xb[kt][:, m * 128:(m + 1) * 128],
                        rhs=wv_sb[:, kt * 192:(kt + 1) * 192],
                        start=(kt == 0), stop=(kt == KT - 1),
                    )
                tt = n * 4 + m
                for hh in range(HPC):
                    nc.vector.tensor_copy(
                        v_sb[:, hh * VSTRIDE + tt * 97:hh * VSTRIDE + tt * 97 + 96],
                        pv[:, hh * 96:(hh + 1) * 96])

        def phase_b(b, ib):
            """Attention + out-projection for q i-block ib of batch b."""
            tok0 = b * T + ib * 512
            ous = []
            for hh in range(HPC):
                ups = ps.tile([97, 512], F32, tag="pS")
                njt = 4 * ib + 4
                for jt in range(njt):
                    sps = ps3.tile([128, 512], F32, tag="pA")
                    jtok = b * T + jt * 128
                    nc.tensor.matmul(
                        sps[:],
                        lhsT=kT_sb[:, hh * NT + jtok:hh * NT + jtok + 128],
                        rhs=qT_sb[:, hh * NT + tok0:hh * NT + tok0 + 512],
                        start=True, stop=True,
                    )
                    es = ep.tile([128, 512], BF16, tag="es")
                    nc.scalar.activation(es[:], sps[:], AF.Exp)
                    if jt >= 4 * ib:
                        s = jt - 4 * ib
                        # keep where (q idx) i - 128*s - j >= 0
                        nc.gpsimd.affine_select(
                            out=es[:], in_=es[:],
                            compare_op=mybir.AluOpType.is_ge,
                            fill=0.0, base=-128 * s, channel_multiplier=-1,
                            pattern=[[1, 512]],
                        )
                    gt = b * 16 + jt
                    nc.tensor.matmul(
                        ups[:],
                        lhsT=v_sb[:, hh * VSTRIDE + gt * 97:hh * VSTRIDE + gt * 97 + 97],
                        rhs=es[:],
                        start=(jt == 0), stop=(jt == njt - 1),
                    )
                zrs = rp.tile([1, 512], F32, tag="row")
                nc.scalar.activation(zrs[:], ups[96:97, :], AF.Abs_reciprocal_sqrt)
                zinv = rp.tile([1, 512], F32, tag="row")
                nc.vector.tensor_mul(zinv[:], zrs[:], zrs[:])
                zb = wk.tile([96, 512], F32, tag="qs")
                nc.gpsimd.partition_broadcast(zb[:], zinv[:])
                ou = op_.tile([96, 512], BF16, tag=f"ou{hh}")
                nc.vector.tensor_mul(ou[:], ups[0:96, :], zb[:])
                ous.append(ou)
            for ct in range(KT):
                ops = ps.tile([128, 512], F32, tag="pV")
                for hh in range(HPC):
                    nc.tensor.matmul(
                        ops[:],
                        lhsT=wo_sb[:, hh * C + ct * 128:hh * C + ct * 128 + 128],
                        rhs=ous[hh][:],
                        start=(hh == 0), stop=(hh == HPC - 1),
                    )
                ob = obp.tile([128, 512], BF16, tag="ob")
                nc.vector.tensor_copy(ob[:], ops[:])
                nc.sync.dma_start(
                    out=outT[ct * 128:(ct + 1) * 128, tok0:tok0 + 512], in_=ob[:])

        # interleave: block n's phase A, then the i-block whose deps it closes
        for n in range(NBLK):
            phase_a(n)
            b, ib = divmod(n, IB_PER_B)
            phase_b(b, ib)

    nc.compile()
    return nc


def _get_nc():
    if "nc" not in _CACHE:
        _CACHE["nc"] = _build()
    return _CACHE["nc"]


# ------------------------------------------------------------------ entrypoint

def _run(inputs, trace=False, **kw):
    nc = _get_nc()
    in_maps = _make_in_maps(**inputs)
    res = run_bass_kernel_spmd(nc, in_maps, core_ids=list(range(NCORES)),
                               trace=trace, **kw)
    acc = np.zeros((C, NT), np.float64)
    for r in res.results:
        acc += r["outT"].astype(np.float64)
    out = np.ascontiguousarray(acc.T.astype(np.float32)).reshape(B, T, C)
    return out, res


def kernel(**inputs) -> np.ndarray:
    out, _ = _run(inputs, trace=False)
    return out
